# revision 20
# baseline (speedup 1.0000x reference)
"""Trainium2 Bass kernel for nn_EncoderTransformerConv (8-core SPMD).

MLP encoder + dense single-head TransformerConv attention + VAE head.
N=8192 nodes sharded 8 ways (1024 query rows/core); K/V all-gathered
in two pipelined halves; attention runs in two j-phases so the second
gather hides under the first phase's compute.

Layouts: activations feature-on-partition ("transposed") so every matmul
contracts over the partition dim with no on-chip input transposes.
float32r (full-rate fp32) for MLP/head; fp8e4m3 for the O(N^2) attention
operands (error contribution ~1e-3 total, gate is 2e-2).
"""
import sys

if '/opt/trn_rl_repo' not in sys.path:
    sys.path.insert(0, '/opt/trn_rl_repo')

import numpy as np
import ml_dtypes
_bf16np = ml_dtypes.bfloat16
import concourse.bass as bass
import concourse.mybir as mybir
import concourse.tile as tile
from concourse import bacc
from concourse.bass_utils import run_bass_kernel_spmd

dt = mybir.dt
F32 = dt.float32
F32R = dt.float32r
ATT = dt.float8e4          # attention operand dtype (e4m3)
BF16 = dt.bfloat16
AF = mybir.ActivationFunctionType

NCORES = 8
N = 8192
NQ = N // NCORES            # 1024 query rows per core
IN_DIM = 1024
HID = 512
E = 256                      # embed dim == head dim
E1 = E + 1                   # V columns + ones column
LAT2 = 32                    # 2 * latent
SCALE = 1.0 / 16.0           # 1/sqrt(E)

# j-tile order: all ranks' first-half tiles (covered by AG part 0), then
# all ranks' second-half tiles (AG part 1).
JPERM = ([r * 8 + jt for r in range(NCORES) for jt in range(4)] +
         [r * 8 + jt for r in range(NCORES) for jt in range(4, 8)])

_cache = {}


def _build():
    nc = bacc.Bacc("TRN2", target_bir_lowering=False, debug=False,
                   num_devices=NCORES)

    # ---- DRAM I/O ----
    xT = nc.dram_tensor("xT", [IN_DIM, NQ], F32R, kind="ExternalInput")
    W1 = nc.dram_tensor("W1", [IN_DIM, HID], F32R, kind="ExternalInput")
    W2 = nc.dram_tensor("W2", [HID, HID], F32R, kind="ExternalInput")
    W3 = nc.dram_tensor("W3", [HID, E], F32R, kind="ExternalInput")
    Wq = nc.dram_tensor("Wq", [E, E], F32R, kind="ExternalInput")
    Wk = nc.dram_tensor("Wk", [E, E], F32R, kind="ExternalInput")
    Wv = nc.dram_tensor("Wv", [E, E], F32R, kind="ExternalInput")
    Wskip = nc.dram_tensor("Wskip", [E, E], F32R, kind="ExternalInput")
    Wself = nc.dram_tensor("Wself", [E, E], F32R, kind="ExternalInput")
    Wctx = nc.dram_tensor("Wctx", [E, E], F32R, kind="ExternalInput")
    Wf1 = nc.dram_tensor("Wf1", [2 * E, HID], F32R, kind="ExternalInput")
    Wf2 = nc.dram_tensor("Wf2", [HID, LAT2], F32R, kind="ExternalInput")
    b1 = nc.dram_tensor("b1", [HID, 1], F32, kind="ExternalInput")
    b2 = nc.dram_tensor("b2", [HID, 1], F32, kind="ExternalInput")
    b3 = nc.dram_tensor("b3", [E, 1], F32, kind="ExternalInput")
    bq = nc.dram_tensor("bq", [E, 1], F32, kind="ExternalInput")
    bk = nc.dram_tensor("bk", [E, 1], F32, kind="ExternalInput")
    bf1 = nc.dram_tensor("bf1", [HID, 1], F32, kind="ExternalInput")
    bf2 = nc.dram_tensor("bf2", [LAT2, 1], F32, kind="ExternalInput")
    bcomb_row = nc.dram_tensor("bcomb_row", [1, E], F32R, kind="ExternalInput")
    ones_row = nc.dram_tensor("ones_row", [1, 128], F32R, kind="ExternalInput")
    ident = nc.dram_tensor("ident", [128, 128], F32R, kind="ExternalInput")
    outT = nc.dram_tensor("outT", [LAT2, NQ], F32, kind="ExternalOutput")

    def chunked_load(sb_t, dram_t, cols):
        nc.sync.dma_start(sb_t[:].rearrange("p (k f) -> p k f", f=cols),
                          dram_t[:].rearrange("(k p) f -> p k f", p=128))

    def bias_load(sb_t, dram_t):
        nc.sync.dma_start(sb_t[:].rearrange("p (k o) -> p k o", o=1),
                          dram_t[:].rearrange("(k p) o -> p k o", p=128))

    with tile.TileContext(nc) as tc:
        with tc.tile_pool(name="persist", bufs=1) as pe_pool, \
             tc.tile_pool(name="ps_mm", bufs=4, space="PSUM") as ps_mm, \
             tc.tile_pool(name="ps_av", bufs=2, space="PSUM") as ps_av, \
             tc.tile_pool(name="ps_tr", bufs=2, space="PSUM") as ps_tr, \
             tc.tile_pool(name="dram", bufs=1, space="DRAM") as dram:

            # ---- persistent SBUF ----
            wq_sb = pe_pool.tile([128, 2 * E], F32R, tag="wq")
            wk_sb = pe_pool.tile([128, 2 * E], F32R, tag="wk")
            wv_sb = pe_pool.tile([128, 2 * E], F32R, tag="wv")
            wskip_sb = pe_pool.tile([128, 2 * E], F32R, tag="wskip")
            wself_sb = pe_pool.tile([128, 2 * E], F32R, tag="wself")
            wctx_sb = pe_pool.tile([128, 2 * E], F32R, tag="wctx")
            wf1_sb = pe_pool.tile([128, 4 * HID], F32R, tag="wf1")
            wf2_sb = pe_pool.tile([128, 4 * LAT2], F32R, tag="wf2")
            b1_sb = pe_pool.tile([128, 4], F32, tag="b1")
            b2_sb = pe_pool.tile([128, 4], F32, tag="b2")
            b3_sb = pe_pool.tile([128, 2], F32, tag="b3")
            bq_sb = pe_pool.tile([128, 2], F32, tag="bq")
            bk_sb = pe_pool.tile([128, 2], F32, tag="bk")
            bf1_sb = pe_pool.tile([128, 4], F32, tag="bf1")
            bf2_sb = pe_pool.tile([128, 1], F32, tag="bf2")
            bcr_sb = pe_pool.tile([1, E], F32R, tag="bcr")
            ones_sb = pe_pool.tile([1, 128], F32R, tag="ones")
            id_sb = pe_pool.tile([128, 128], F32R, tag="ident")
            ht_sb = pe_pool.tile([128, 2 * NQ], F32R, tag="ht")
            qt_sb = pe_pool.tile([128, 2 * NQ], ATT, tag="qt")
            skip_sb = pe_pool.tile([128, 8 * E], F32, tag="skip")
            hselft_sb = pe_pool.tile([128, 2 * NQ], F32R, tag="hselft")
            hfuset_sb = pe_pool.tile([128, 2 * NQ], F32R, tag="hfuset")
            hctxt_sb = pe_pool.tile([128, 2 * NQ], F32R, tag="hctxt")
            recip_sb = pe_pool.tile([128, 8], F32, tag="recip")
            attn_sb = pe_pool.tile([128, 8 * E1], F32, tag="attn")
            outt_sb = pe_pool.tile([128, NQ], F32, tag="outt")

            # AG bounce buffers: per half, [c0 512 | c1 512 | V(4x257)] fp8
            AGW = 2 * 512 + 4 * E1        # 2052 fp8 columns
            ag_in0 = dram.tile([128, AGW], ATT, tag="agi0")
            ag_in1 = dram.tile([128, AGW], ATT, tag="agi1")
            ag_in = [ag_in0, ag_in1]
            ag_out = [nc.dram_tensor(f"ag_out{h}", [NCORES * 128, AGW],
                                     ATT, kind="Internal",
                                     addr_space="Shared")
                      for h in range(2)]

            # ======== phase 1: MLP + projections + split AllGather ========
            with tc.tile_pool(name="mlp", bufs=1) as mlp_pool, \
                 tc.tile_pool(name="xt", bufs=16) as xt_pool:
                w1_sb = mlp_pool.tile([128, 8 * HID], F32R, tag="w1")
                w2_sb = mlp_pool.tile([128, 4 * HID], F32R, tag="w2")
                w3_sb = mlp_pool.tile([128, 4 * E], F32R, tag="w3")
                h1t_sb = mlp_pool.tile([128, 4 * NQ], F32R, tag="h1t")
                h2t_sb = mlp_pool.tile([128, 4 * NQ], F32R, tag="h2t")
                kts_sb = mlp_pool.tile([128, 2 * NQ], ATT, tag="kts")
                vs_sb = mlp_pool.tile([128, 8 * E1], ATT, tag="vs")

                def load_x(i2, interleave_w1):
                    # per-chunk loads; optionally interleave W1 chunk loads
                    # so the k-outer h1 matmuls can start after chunk 0
                    xts = []
                    for k in range(8):
                        if interleave_w1:
                            nc.sync.dma_start(
                                w1_sb[:, k * HID:(k + 1) * HID],
                                W1[k * 128:(k + 1) * 128, :])
                        xt_t = xt_pool.tile([128, 512], F32R, tag="xt")
                        nc.sync.dma_start(
                            xt_t[:],
                            xT[k * 128:(k + 1) * 128,
                               i2 * 512:(i2 + 1) * 512])
                        xts.append(xt_t)
                    return xts

                def h1_half(i2, xts):
                    # k-outer: 4 PSUM groups held; streams x chunk-by-chunk
                    ps_h1 = []
                    for _f in range(4):
                        ph1_t = ps_mm.tile([128, 512], F32, tag="mm")
                        ps_h1.append(ph1_t)
                    for k in range(8):
                        for f in range(4):
                            nc.tensor.matmul(
                                ps_h1[f][:],
                                w1_sb[:, k * HID + f * 128:
                                      k * HID + (f + 1) * 128],
                                xts[k][:], start=(k == 0), stop=(k == 7))
                    for f in range(4):
                        nc.scalar.activation(
                            h1t_sb[:, f * NQ + i2 * 512:
                                   f * NQ + i2 * 512 + 512],
                            ps_h1[f][:], AF.Relu, bias=b1_sb[:, f:f + 1])

                def mlp_half(i2):
                    for f in range(4):
                        p = ps_mm.tile([128, 512], F32, tag="mm")
                        for k in range(4):
                            nc.tensor.matmul(
                                p[:],
                                w2_sb[:, k * HID + f * 128:
                                      k * HID + (f + 1) * 128],
                                h1t_sb[:, k * NQ + i2 * 512:
                                       k * NQ + i2 * 512 + 512],
                                start=(k == 0), stop=(k == 3))
                        nc.scalar.activation(
                            h2t_sb[:, f * NQ + i2 * 512:
                                   f * NQ + i2 * 512 + 512],
                            p[:], AF.Relu, bias=b2_sb[:, f:f + 1])
                    for c in range(2):
                        p = ps_mm.tile([128, 512], F32, tag="mm")
                        for k in range(4):
                            nc.tensor.matmul(
                                p[:],
                                w3_sb[:, k * E + c * 128:
                                      k * E + (c + 1) * 128],
                                h2t_sb[:, k * NQ + i2 * 512:
                                       k * NQ + i2 * 512 + 512],
                                start=(k == 0), stop=(k == 3))
                        nc.scalar.activation(
                            ht_sb[:, c * NQ + i2 * 512:
                                  c * NQ + i2 * 512 + 512],
                            p[:], AF.Relu, bias=b3_sb[:, c:c + 1])

                def kv_half(i2):
                    # K^T columns for this half (+bk), fp8
                    for c2 in range(2):
                        p = ps_mm.tile([128, 512], F32, tag="mm")
                        for c in range(2):
                            nc.tensor.matmul(
                                p[:],
                                wk_sb[:, c * E + c2 * 128:
                                      c * E + (c2 + 1) * 128],
                                ht_sb[:, c * NQ + i2 * 512:
                                      c * NQ + i2 * 512 + 512],
                                start=(c == 0), stop=(c == 1))
                        nc.vector.tensor_scalar_add(
                            kts_sb[:, c2 * NQ + i2 * 512:
                                   c2 * NQ + i2 * 512 + 512],
                            p[:], bk_sb[:, c2:c2 + 1])
                    # V rows for this half (bias bv folded into bcomb)
                    for jt in range(i2 * 4, i2 * 4 + 4):
                        p = ps_mm.tile([128, 256], F32, tag="mm")
                        for c in range(2):
                            nc.tensor.matmul(
                                p[:],
                                ht_sb[:, c * NQ + jt * 128:
                                      c * NQ + (jt + 1) * 128],
                                wv_sb[:, c * E:(c + 1) * E],
                                start=(c == 0), stop=(c == 1))
                        nc.vector.tensor_copy(
                            vs_sb[:, jt * E1:jt * E1 + E], p[:])
                    nc.vector.memset(
                        vs_sb[:, i2 * 4 * E1:(i2 * 4 + 4) * E1].rearrange(
                            "p (j c) -> p j c", c=E1)[:, :, E:E1], 1.0)
                    # bounce + AllGather this half
                    nc.scalar.dma_start(ag_in[i2][:, 0:512],
                                        kts_sb[:, i2 * 512:i2 * 512 + 512])
                    nc.scalar.dma_start(ag_in[i2][:, 512:1024],
                                        kts_sb[:, NQ + i2 * 512:
                                               NQ + i2 * 512 + 512])
                    nc.scalar.dma_start(ag_in[i2][:, 1024:AGW],
                                        vs_sb[:, i2 * 4 * E1:(i2 * 4 + 4) * E1])
                    nc.gpsimd.collective_compute(
                        "AllGather", mybir.AluOpType.bypass,
                        replica_groups=[list(range(NCORES))],
                        ins=[ag_in[i2][:].opt()],
                        outs=[ag_out[i2][:].opt()],
                    )

                # identity first (tiny), then warmup matmuls keep the
                # PE HAM clock-gate busy (-> 2.4 GHz) while W1/x stream in
                nc.sync.dma_start(id_sb[:], ident[:])
                warm_ps = ps_av.tile([128, E1], F32, tag="av")
                for _w in range(20):
                    nc.tensor.matmul(warm_ps[:, 0:128], id_sb[:], id_sb[:],
                                     start=True, stop=True)
                bias_load(b1_sb, b1)
                xts0 = load_x(0, interleave_w1=True)
                # remaining MLP weights behind W1/x0 in queue order
                chunked_load(w2_sb, W2, HID)
                chunked_load(w3_sb, W3, E)
                chunked_load(wk_sb, Wk, E)
                chunked_load(wv_sb, Wv, E)
                bias_load(b2_sb, b2)
                bias_load(b3_sb, b3)
                bias_load(bk_sb, bk)
                h1_half(0, xts0)
                xts1 = load_x(1, interleave_w1=False)
                mlp_half(0)
                kv_half(0)
                # attention/head weights prefetched during half-0 compute
                # (AG bounces ride the ACT HWDGE queue, so these sync-queue
                # loads no longer delay the collective)
                chunked_load(wq_sb, Wq, E)
                chunked_load(wskip_sb, Wskip, E)
                chunked_load(wself_sb, Wself, E)
                chunked_load(wctx_sb, Wctx, E)
                chunked_load(wf1_sb, Wf1, HID)
                chunked_load(wf2_sb, Wf2, LAT2)
                bias_load(bq_sb, bq)
                bias_load(bf1_sb, bf1)
                nc.sync.dma_start(bf2_sb[0:LAT2, :], bf2[:])
                nc.sync.dma_start(bcr_sb[:], bcomb_row[:])
                nc.sync.dma_start(ones_sb[:], ones_row[:])
                h1_half(1, xts1)
                mlp_half(1)
                kv_half(1)

                # ---- overlap with AG: Q^T (+bq, fp8) ----
                for c2 in range(2):
                    for i2 in range(2):
                        p = ps_mm.tile([128, 512], F32, tag="mm")
                        for c in range(2):
                            nc.tensor.matmul(
                                p[:],
                                wq_sb[:, c * E + c2 * 128:
                                      c * E + (c2 + 1) * 128],
                                ht_sb[:, c * NQ + i2 * 512:
                                      c * NQ + i2 * 512 + 512],
                                start=(c == 0), stop=(c == 1))
                        nc.vector.tensor_scalar_add(
                            qt_sb[:, c2 * NQ + i2 * 512:
                                  c2 * NQ + i2 * 512 + 512],
                            p[:], bq_sb[:, c2:c2 + 1])

                # ---- overlap with AG: skip = H@Wskip + (bskip+bv) ----
                for it in range(8):
                    p = ps_mm.tile([128, 256], F32, tag="mm")
                    for c in range(2):
                        nc.tensor.matmul(
                            p[:],
                            ht_sb[:, c * NQ + it * 128:
                                  c * NQ + (it + 1) * 128],
                            wskip_sb[:, c * E:(c + 1) * E],
                            start=(c == 0), stop=False)
                    nc.tensor.matmul(p[:], ones_sb[:], bcr_sb[:],
                                     start=False, stop=True)
                    nc.vector.tensor_copy(skip_sb[:, it * E:(it + 1) * E],
                                          p[:])

                # ---- overlap with AG: h_selfT = Wself^T @ H^T ----
                for c2 in range(2):
                    for i2 in range(2):
                        p = ps_mm.tile([128, 512], F32, tag="mm")
                        for c in range(2):
                            nc.tensor.matmul(
                                p[:],
                                wself_sb[:, c * E + c2 * 128:
                                         c * E + (c2 + 1) * 128],
                                ht_sb[:, c * NQ + i2 * 512:
                                      c * NQ + i2 * 512 + 512],
                                start=(c == 0), stop=(c == 1))
                        nc.vector.tensor_copy(
                            hselft_sb[:, c2 * NQ + i2 * 512:
                                      c2 * NQ + i2 * 512 + 512], p[:])

            # ======== phase 2: attention (mlp pool released) ========
            with tc.tile_pool(name="attn", bufs=1) as at_pool:
                kt_sb = at_pool.tile([128, 2 * N], ATT, tag="kt")
                vones_sb = at_pool.tile([128, 64 * E1], ATT, tag="vones")

                def load_gathered(h):
                    for r in range(NCORES):
                        for c in range(2):
                            nc.sync.dma_start(
                                kt_sb[:, c * N + r * NQ + h * 512:
                                      c * N + r * NQ + h * 512 + 512],
                                ag_out[h][r * 128:(r + 1) * 128,
                                          c * 512:(c + 1) * 512])
                    for r in range(NCORES):
                        nc.sync.dma_start(
                            vones_sb[:, (r * 8 + h * 4) * E1:
                                     (r * 8 + h * 4 + 4) * E1],
                            ag_out[h][r * 128:(r + 1) * 128, 1024:AGW])

                load_gathered(0)
                load_gathered(1)

                def st_quarter(b, ph, pt_sb):
                    # S^T+exp for 16 pairs (32 j-tiles) of phase ph
                    for pr in range(16):
                        p_s = ps_mm.tile([128, 512], F32, tag="mm")
                        for half in range(2):
                            j = JPERM[ph * 32 + 2 * pr + half]
                            for c in range(2):
                                nc.tensor.matmul(
                                    p_s[:, half * 256:half * 256 + 256],
                                    kt_sb[:, c * N + j * 128:
                                          c * N + (j + 1) * 128],
                                    qt_sb[:, c * NQ + b * 256:
                                          c * NQ + (b + 1) * 256],
                                    start=(c == 0), stop=(c == 1))
                        nc.scalar.activation(
                            pt_sb[:, pr * 512:(pr + 1) * 512],
                            p_s[:], AF.Exp, scale=SCALE)

                def av_quarter(b, ph, pt_sb):
                    # AV partial over this phase's 32 j-tiles
                    for ic in range(2):
                        it = b * 2 + ic
                        p_av = ps_av.tile([128, E1], F32, tag="av")
                        for q in range(32):
                            jg = JPERM[ph * 32 + q]
                            nc.tensor.matmul(
                                p_av[:],
                                pt_sb[:, q * E + ic * 128:
                                      q * E + ic * 128 + 128],
                                vones_sb[:, jg * E1:(jg + 1) * E1],
                                start=(q == 0), stop=(q == 31))
                        sl = attn_sb[:, it * E1:(it + 1) * E1]
                        if ph == 0:
                            nc.vector.tensor_copy(sl, p_av[:])
                        else:
                            nc.vector.tensor_add(sl, sl, p_av[:])

                hiddent_sb = at_pool.tile([128, 4 * NQ], F32R, tag="hiddent")

                def finalize(b, hf_pool):
                    for ic in range(2):
                        it = b * 2 + ic
                        nc.vector.reciprocal(
                            recip_sb[:, it:it + 1],
                            attn_sb[:, it * E1 + E:(it + 1) * E1])
                        hf_t = hf_pool.tile([128, E], F32R, tag="hf")
                        nc.vector.scalar_tensor_tensor(
                            hf_t[:], attn_sb[:, it * E1:it * E1 + E],
                            recip_sb[:, it:it + 1],
                            skip_sb[:, it * E:(it + 1) * E],
                            op0=mybir.AluOpType.mult,
                            op1=mybir.AluOpType.add)
                        for c in range(2):
                            p_tr = ps_tr.tile([128, 128], F32R, tag="tr")
                            nc.tensor.transpose(
                                p_tr[:], hf_t[:, c * 128:(c + 1) * 128],
                                id_sb[:])
                            nc.vector.tensor_copy(
                                hfuset_sb[:, c * NQ + it * 128:
                                          c * NQ + (it + 1) * 128],
                                p_tr[:])

                def head_half(i2):
                    # h_fuseT = Wctx^T @ H_fuse^T for this query half
                    for c2 in range(2):
                        p = ps_mm.tile([128, 512], F32, tag="mm")
                        for c in range(2):
                            nc.tensor.matmul(
                                p[:],
                                wctx_sb[:, c * E + c2 * 128:
                                        c * E + (c2 + 1) * 128],
                                hfuset_sb[:, c * NQ + i2 * 512:
                                          c * NQ + i2 * 512 + 512],
                                start=(c == 0), stop=(c == 1))
                        nc.vector.tensor_copy(
                            hctxt_sb[:, c2 * NQ + i2 * 512:
                                     c2 * NQ + i2 * 512 + 512], p[:])
                    for f in range(4):
                        p = ps_mm.tile([128, 512], F32, tag="mm")
                        for kc in range(4):
                            rhs_sb = hselft_sb if kc < 2 else hctxt_sb
                            nc.tensor.matmul(
                                p[:],
                                wf1_sb[:, kc * HID + f * 128:
                                       kc * HID + (f + 1) * 128],
                                rhs_sb[:, (kc % 2) * NQ + i2 * 512:
                                       (kc % 2) * NQ + i2 * 512 + 512],
                                start=(kc == 0), stop=(kc == 3))
                        nc.scalar.activation(
                            hiddent_sb[:, f * NQ + i2 * 512:
                                       f * NQ + i2 * 512 + 512],
                            p[:], AF.Relu, bias=bf1_sb[:, f:f + 1])
                    p = ps_mm.tile([128, 512], F32, tag="mm")
                    for kc in range(4):
                        nc.tensor.matmul(
                            p[0:LAT2, :],
                            wf2_sb[:, kc * LAT2:(kc + 1) * LAT2],
                            hiddent_sb[:, kc * NQ + i2 * 512:
                                       kc * NQ + i2 * 512 + 512],
                            start=(kc == 0), stop=(kc == 3))
                    nc.scalar.activation(
                        outt_sb[0:LAT2, i2 * 512:i2 * 512 + 512],
                        p[0:LAT2, :], AF.Identity, bias=bf2_sb[0:LAT2, :])
                    nc.sync.dma_start(outT[:, i2 * 512:i2 * 512 + 512],
                                      outt_sb[0:LAT2, i2 * 512:i2 * 512 + 512])

                with tc.tile_pool(name="pt", bufs=2) as pt_pool, \
                     tc.tile_pool(name="hf", bufs=3) as hf_pool:
                    # phase A: all blocks on AG0's j-tiles (hides AG1)
                    for b in range(4):
                        pt_sb = pt_pool.tile([128, 32 * E], ATT, tag="pt")
                        st_quarter(b, 0, pt_sb)
                        av_quarter(b, 0, pt_sb)
                    # phase B: AG1's j-tiles + finalize; head overlapped
                    # per query-half as its H_fuse^T tiles complete
                    for b in range(4):
                        pt_sb = pt_pool.tile([128, 32 * E], ATT, tag="pt")
                        st_quarter(b, 1, pt_sb)
                        av_quarter(b, 1, pt_sb)
                        finalize(b, hf_pool)
                        if b == 1:
                            head_half(0)
                    head_half(1)

    nc.compile()
    return nc


def _get_nc():
    if "nc" not in _cache:
        _cache["nc"] = _build()
    return _cache["nc"]


def kernel(x, W1, b1, W2, b2, W3, b3, Wq, bq, Wk, bk, Wv, bv,
           Wskip, bskip, Wself, Wctx, Wf1, bf1, Wf2, bf2,
           _trace=False, _tmpdir=None):
    nc = _get_nc()
    f32 = np.float32
    x = np.asarray(x, f32)
    xT_full = np.ascontiguousarray(x.T)                      # [1024, 8192]
    col = lambda v: np.ascontiguousarray(np.asarray(v, f32).reshape(-1, 1))
    bcomb = (np.asarray(bskip, f32) + np.asarray(bv, f32)).reshape(1, -1)
    shared = {
        "W1": np.ascontiguousarray(W1, f32),
        "W2": np.ascontiguousarray(W2, f32),
        "W3": np.ascontiguousarray(W3, f32),
        "Wq": np.ascontiguousarray(Wq, f32),
        "Wk": np.ascontiguousarray(Wk, f32),
        "Wv": np.ascontiguousarray(Wv, f32),
        "Wskip": np.ascontiguousarray(Wskip, f32),
        "Wself": np.ascontiguousarray(Wself, f32),
        "Wctx": np.ascontiguousarray(Wctx, f32),
        "Wf1": np.ascontiguousarray(Wf1, f32),
        "Wf2": np.ascontiguousarray(Wf2, f32),
        "b1": col(b1), "b2": col(b2), "b3": col(b3),
        "bq": col(bq), "bk": col(bk),
        "bf1": col(bf1), "bf2": col(bf2),
        "bcomb_row": np.ascontiguousarray(bcomb),
        "ones_row": np.ones((1, 128), f32),
        "ident": np.eye(128, dtype=f32),
    }
    in_maps = []
    for r in range(NCORES):
        m = dict(shared)
        m["xT"] = np.ascontiguousarray(xT_full[:, r * NQ:(r + 1) * NQ])
        in_maps.append(m)

    kwargs = {}
    if _trace:
        kwargs = dict(trace=True, tmpdir=_tmpdir)
    res = run_bass_kernel_spmd(nc, in_maps, core_ids=list(range(NCORES)),
                               **kwargs)
    out = np.empty((N, LAT2), f32)
    for r in range(NCORES):
        out[r * NQ:(r + 1) * NQ, :] = res.results[r]["outT"].T
    mu = np.ascontiguousarray(out[:, :LAT2 // 2])
    logvar = np.ascontiguousarray(out[:, LAT2 // 2:])
    if _trace:
        return (mu, logvar), res
    return mu, logvar


# revision 21
# speedup vs baseline: 1.0202x; 1.0202x over previous
"""Trainium2 Bass kernel for nn_EncoderTransformerConv (8-core SPMD).

MLP encoder + dense single-head TransformerConv attention + VAE head.
N=8192 nodes sharded 8 ways (1024 query rows/core); K/V all-gathered
in two pipelined halves; attention runs in two j-phases so the second
gather hides under the first phase's compute.

Layouts: activations feature-on-partition ("transposed") so every matmul
contracts over the partition dim with no on-chip input transposes.
float32r (full-rate fp32) for MLP/head; fp8e4m3 for the O(N^2) attention
operands (error contribution ~1e-3 total, gate is 2e-2).
"""
import sys

if '/opt/trn_rl_repo' not in sys.path:
    sys.path.insert(0, '/opt/trn_rl_repo')

import numpy as np
import ml_dtypes
_bf16np = ml_dtypes.bfloat16
import concourse.bass as bass
import concourse.mybir as mybir
import concourse.tile as tile
from concourse import bacc
from concourse.bass_utils import run_bass_kernel_spmd

dt = mybir.dt
F32 = dt.float32
F32R = dt.float32r
ATT = dt.float8e4          # attention operand dtype (e4m3)
BF16 = dt.bfloat16
AF = mybir.ActivationFunctionType

NCORES = 8
N = 8192
NQ = N // NCORES            # 1024 query rows per core
IN_DIM = 1024
HID = 512
E = 256                      # embed dim == head dim
E1 = E + 1                   # V columns + ones column
LAT2 = 32                    # 2 * latent
SCALE = 1.0 / 16.0           # 1/sqrt(E)

# j-tile order: all ranks' first-half tiles (covered by AG part 0), then
# all ranks' second-half tiles (AG part 1).
JPERM = ([r * 8 + jt for r in range(NCORES) for jt in range(4)] +
         [r * 8 + jt for r in range(NCORES) for jt in range(4, 8)])

_cache = {}


def _build():
    nc = bacc.Bacc("TRN2", target_bir_lowering=False, debug=False,
                   num_devices=NCORES)

    # ---- DRAM I/O ----
    xT = nc.dram_tensor("xT", [IN_DIM, NQ], F32R, kind="ExternalInput")
    W1 = nc.dram_tensor("W1", [IN_DIM, HID], F32R, kind="ExternalInput")
    W2 = nc.dram_tensor("W2", [HID, HID], F32R, kind="ExternalInput")
    W3 = nc.dram_tensor("W3", [HID, E], F32R, kind="ExternalInput")
    Wq = nc.dram_tensor("Wq", [E, E], F32R, kind="ExternalInput")
    Wk = nc.dram_tensor("Wk", [E, E], F32R, kind="ExternalInput")
    Wv = nc.dram_tensor("Wv", [E, E], F32R, kind="ExternalInput")
    Wskip = nc.dram_tensor("Wskip", [E, E], F32R, kind="ExternalInput")
    Wself = nc.dram_tensor("Wself", [E, E], F32R, kind="ExternalInput")
    Wctx = nc.dram_tensor("Wctx", [E, E], F32R, kind="ExternalInput")
    Wf1 = nc.dram_tensor("Wf1", [2 * E, HID], F32R, kind="ExternalInput")
    Wf2 = nc.dram_tensor("Wf2", [HID, LAT2], F32R, kind="ExternalInput")
    b1 = nc.dram_tensor("b1", [HID, 1], F32, kind="ExternalInput")
    b2 = nc.dram_tensor("b2", [HID, 1], F32, kind="ExternalInput")
    b3 = nc.dram_tensor("b3", [E, 1], F32, kind="ExternalInput")
    bq = nc.dram_tensor("bq", [E, 1], F32, kind="ExternalInput")
    bk = nc.dram_tensor("bk", [E, 1], F32, kind="ExternalInput")
    bf1 = nc.dram_tensor("bf1", [HID, 1], F32, kind="ExternalInput")
    bf2 = nc.dram_tensor("bf2", [LAT2, 1], F32, kind="ExternalInput")
    bcomb_row = nc.dram_tensor("bcomb_row", [1, E], F32R, kind="ExternalInput")
    ones_row = nc.dram_tensor("ones_row", [1, 128], F32R, kind="ExternalInput")
    ident = nc.dram_tensor("ident", [128, 128], F32R, kind="ExternalInput")
    outT = nc.dram_tensor("outT", [LAT2, NQ], F32, kind="ExternalOutput")

    def chunked_load(sb_t, dram_t, cols):
        nc.sync.dma_start(sb_t[:].rearrange("p (k f) -> p k f", f=cols),
                          dram_t[:].rearrange("(k p) f -> p k f", p=128))

    def bias_load(sb_t, dram_t):
        nc.sync.dma_start(sb_t[:].rearrange("p (k o) -> p k o", o=1),
                          dram_t[:].rearrange("(k p) o -> p k o", p=128))

    with tile.TileContext(nc) as tc:
        with tc.tile_pool(name="persist", bufs=1) as pe_pool, \
             tc.tile_pool(name="ps_mm", bufs=4, space="PSUM") as ps_mm, \
             tc.tile_pool(name="ps_av", bufs=2, space="PSUM") as ps_av, \
             tc.tile_pool(name="ps_tr", bufs=2, space="PSUM") as ps_tr, \
             tc.tile_pool(name="dram", bufs=1, space="DRAM") as dram:

            # ---- persistent SBUF ----
            wq_sb = pe_pool.tile([128, 2 * E], F32R, tag="wq")
            wk_sb = pe_pool.tile([128, 2 * E], F32R, tag="wk")
            wv_sb = pe_pool.tile([128, 2 * E], F32R, tag="wv")
            wskip_sb = pe_pool.tile([128, 2 * E], F32R, tag="wskip")
            wself_sb = pe_pool.tile([128, 2 * E], F32R, tag="wself")
            wctx_sb = pe_pool.tile([128, 2 * E], F32R, tag="wctx")
            wf1_sb = pe_pool.tile([128, 4 * HID], F32R, tag="wf1")
            wf2_sb = pe_pool.tile([128, 4 * LAT2], F32R, tag="wf2")
            b1_sb = pe_pool.tile([128, 4], F32, tag="b1")
            b2_sb = pe_pool.tile([128, 4], F32, tag="b2")
            b3_sb = pe_pool.tile([128, 2], F32, tag="b3")
            bq_sb = pe_pool.tile([128, 2], F32, tag="bq")
            bk_sb = pe_pool.tile([128, 2], F32, tag="bk")
            bf1_sb = pe_pool.tile([128, 4], F32, tag="bf1")
            bf2_sb = pe_pool.tile([128, 1], F32, tag="bf2")
            bcr_sb = pe_pool.tile([1, E], F32R, tag="bcr")
            ones_sb = pe_pool.tile([1, 128], F32R, tag="ones")
            id_sb = pe_pool.tile([128, 128], F32R, tag="ident")
            ht_sb = pe_pool.tile([128, 2 * NQ], F32R, tag="ht")
            qt_sb = pe_pool.tile([128, 2 * NQ], ATT, tag="qt")
            skip_sb = pe_pool.tile([128, 8 * E], F32, tag="skip")
            hselft_sb = pe_pool.tile([128, 2 * NQ], F32R, tag="hselft")
            hfuset_sb = pe_pool.tile([128, 2 * NQ], F32R, tag="hfuset")
            hctxt_sb = pe_pool.tile([128, 2 * NQ], F32R, tag="hctxt")
            recip_sb = pe_pool.tile([128, 8], F32, tag="recip")
            attn_sb = pe_pool.tile([128, 8 * E1], F32, tag="attn")
            outt_sb = pe_pool.tile([128, NQ], F32, tag="outt")

            # AG bounce buffers: per half, [c0 512 | c1 512 | V(4x257)] fp8
            AGW = 2 * 512 + 4 * E1        # 2052 fp8 columns
            ag_in0 = dram.tile([128, AGW], ATT, tag="agi0")
            ag_in1 = dram.tile([128, AGW], ATT, tag="agi1")
            ag_in = [ag_in0, ag_in1]
            ag_out = [nc.dram_tensor(f"ag_out{h}", [NCORES * 128, AGW],
                                     ATT, kind="Internal",
                                     addr_space="Shared")
                      for h in range(2)]

            # ======== phase 1: MLP + projections + split AllGather ========
            with tc.tile_pool(name="mlp", bufs=1) as mlp_pool, \
                 tc.tile_pool(name="xt", bufs=16) as xt_pool:
                w1_sb = mlp_pool.tile([128, 8 * HID], F32R, tag="w1")
                w2_sb = mlp_pool.tile([128, 4 * HID], F32R, tag="w2")
                w3_sb = mlp_pool.tile([128, 4 * E], F32R, tag="w3")
                h1t_sb = mlp_pool.tile([128, 4 * NQ], F32R, tag="h1t")
                h2t_sb = mlp_pool.tile([128, 4 * NQ], F32R, tag="h2t")
                kts_sb = mlp_pool.tile([128, 2 * NQ], ATT, tag="kts")
                vs_sb = mlp_pool.tile([128, 8 * E1], ATT, tag="vs")

                def load_x(i2, interleave_w1):
                    # per-chunk loads; optionally interleave W1 chunk loads
                    # so the k-outer h1 matmuls can start after chunk 0
                    xts = []
                    for k in range(8):
                        if interleave_w1:
                            nc.sync.dma_start(
                                w1_sb[:, k * HID:(k + 1) * HID],
                                W1[k * 128:(k + 1) * 128, :])
                        xt_t = xt_pool.tile([128, 512], F32R, tag="xt")
                        nc.sync.dma_start(
                            xt_t[:],
                            xT[k * 128:(k + 1) * 128,
                               i2 * 512:(i2 + 1) * 512])
                        xts.append(xt_t)
                    return xts

                def h1_half(i2, xts):
                    # k-outer: 4 PSUM groups held; streams x chunk-by-chunk
                    ps_h1 = []
                    for _f in range(4):
                        ph1_t = ps_mm.tile([128, 512], F32, tag="mm")
                        ps_h1.append(ph1_t)
                    for k in range(8):
                        for f in range(4):
                            nc.tensor.matmul(
                                ps_h1[f][:],
                                w1_sb[:, k * HID + f * 128:
                                      k * HID + (f + 1) * 128],
                                xts[k][:], start=(k == 0), stop=(k == 7))
                    for f in range(4):
                        nc.scalar.activation(
                            h1t_sb[:, f * NQ + i2 * 512:
                                   f * NQ + i2 * 512 + 512],
                            ps_h1[f][:], AF.Relu, bias=b1_sb[:, f:f + 1])

                def mlp_half(i2):
                    for f in range(4):
                        p = ps_mm.tile([128, 512], F32, tag="mm")
                        for k in range(4):
                            nc.tensor.matmul(
                                p[:],
                                w2_sb[:, k * HID + f * 128:
                                      k * HID + (f + 1) * 128],
                                h1t_sb[:, k * NQ + i2 * 512:
                                       k * NQ + i2 * 512 + 512],
                                start=(k == 0), stop=(k == 3))
                        nc.scalar.activation(
                            h2t_sb[:, f * NQ + i2 * 512:
                                   f * NQ + i2 * 512 + 512],
                            p[:], AF.Relu, bias=b2_sb[:, f:f + 1])
                    for c in range(2):
                        p = ps_mm.tile([128, 512], F32, tag="mm")
                        for k in range(4):
                            nc.tensor.matmul(
                                p[:],
                                w3_sb[:, k * E + c * 128:
                                      k * E + (c + 1) * 128],
                                h2t_sb[:, k * NQ + i2 * 512:
                                       k * NQ + i2 * 512 + 512],
                                start=(k == 0), stop=(k == 3))
                        nc.scalar.activation(
                            ht_sb[:, c * NQ + i2 * 512:
                                  c * NQ + i2 * 512 + 512],
                            p[:], AF.Relu, bias=b3_sb[:, c:c + 1])

                def kv_half(i2):
                    # K^T columns for this half (+bk), fp8
                    for c2 in range(2):
                        p = ps_mm.tile([128, 512], F32, tag="mm")
                        for c in range(2):
                            nc.tensor.matmul(
                                p[:],
                                wk_sb[:, c * E + c2 * 128:
                                      c * E + (c2 + 1) * 128],
                                ht_sb[:, c * NQ + i2 * 512:
                                      c * NQ + i2 * 512 + 512],
                                start=(c == 0), stop=(c == 1))
                        nc.vector.tensor_scalar_add(
                            kts_sb[:, c2 * NQ + i2 * 512:
                                   c2 * NQ + i2 * 512 + 512],
                            p[:], bk_sb[:, c2:c2 + 1])
                    # V rows for this half (bias bv folded into bcomb)
                    for jt in range(i2 * 4, i2 * 4 + 4):
                        p = ps_mm.tile([128, 256], F32, tag="mm")
                        for c in range(2):
                            nc.tensor.matmul(
                                p[:],
                                ht_sb[:, c * NQ + jt * 128:
                                      c * NQ + (jt + 1) * 128],
                                wv_sb[:, c * E:(c + 1) * E],
                                start=(c == 0), stop=(c == 1))
                        nc.vector.tensor_copy(
                            vs_sb[:, jt * E1:jt * E1 + E], p[:])
                    nc.vector.memset(
                        vs_sb[:, i2 * 4 * E1:(i2 * 4 + 4) * E1].rearrange(
                            "p (j c) -> p j c", c=E1)[:, :, E:E1], 1.0)
                    # bounce + AllGather this half
                    nc.scalar.dma_start(ag_in[i2][:, 0:512],
                                        kts_sb[:, i2 * 512:i2 * 512 + 512])
                    nc.scalar.dma_start(ag_in[i2][:, 512:1024],
                                        kts_sb[:, NQ + i2 * 512:
                                               NQ + i2 * 512 + 512])
                    nc.scalar.dma_start(ag_in[i2][:, 1024:AGW],
                                        vs_sb[:, i2 * 4 * E1:(i2 * 4 + 4) * E1])
                    nc.gpsimd.collective_compute(
                        "AllGather", mybir.AluOpType.bypass,
                        replica_groups=[list(range(NCORES))],
                        ins=[ag_in[i2][:].opt()],
                        outs=[ag_out[i2][:].opt()],
                    )

                # identity first (tiny), then warmup matmuls keep the
                # PE HAM clock-gate busy (-> 2.4 GHz) while W1/x stream in
                nc.sync.dma_start(id_sb[:], ident[:])
                warm_ps = ps_av.tile([128, E1], F32, tag="av")
                for _w in range(20):
                    nc.tensor.matmul(warm_ps[:, 0:128], id_sb[:], id_sb[:],
                                     start=True, stop=True)
                bias_load(b1_sb, b1)
                xts0 = load_x(0, interleave_w1=True)
                # remaining MLP weights behind W1/x0 in queue order
                chunked_load(w2_sb, W2, HID)
                chunked_load(w3_sb, W3, E)
                chunked_load(wk_sb, Wk, E)
                chunked_load(wv_sb, Wv, E)
                bias_load(b2_sb, b2)
                bias_load(b3_sb, b3)
                bias_load(bk_sb, bk)
                h1_half(0, xts0)
                mlp_half(0)
                kv_half(0)
                xts1 = load_x(1, interleave_w1=False)
                # attention/head weights prefetched during half-0 compute
                chunked_load(wq_sb, Wq, E)
                chunked_load(wskip_sb, Wskip, E)
                chunked_load(wself_sb, Wself, E)
                chunked_load(wctx_sb, Wctx, E)
                chunked_load(wf1_sb, Wf1, HID)
                chunked_load(wf2_sb, Wf2, LAT2)
                bias_load(bq_sb, bq)
                bias_load(bf1_sb, bf1)
                nc.sync.dma_start(bf2_sb[0:LAT2, :], bf2[:])
                nc.sync.dma_start(bcr_sb[:], bcomb_row[:])
                nc.sync.dma_start(ones_sb[:], ones_row[:])
                h1_half(1, xts1)
                mlp_half(1)
                kv_half(1)

                # ---- overlap with AG: Q^T (+bq, fp8) ----
                for c2 in range(2):
                    for i2 in range(2):
                        p = ps_mm.tile([128, 512], F32, tag="mm")
                        for c in range(2):
                            nc.tensor.matmul(
                                p[:],
                                wq_sb[:, c * E + c2 * 128:
                                      c * E + (c2 + 1) * 128],
                                ht_sb[:, c * NQ + i2 * 512:
                                      c * NQ + i2 * 512 + 512],
                                start=(c == 0), stop=(c == 1))
                        nc.vector.tensor_scalar_add(
                            qt_sb[:, c2 * NQ + i2 * 512:
                                  c2 * NQ + i2 * 512 + 512],
                            p[:], bq_sb[:, c2:c2 + 1])

                # ---- overlap with AG: skip = H@Wskip + (bskip+bv) ----
                for it in range(8):
                    p = ps_mm.tile([128, 256], F32, tag="mm")
                    for c in range(2):
                        nc.tensor.matmul(
                            p[:],
                            ht_sb[:, c * NQ + it * 128:
                                  c * NQ + (it + 1) * 128],
                            wskip_sb[:, c * E:(c + 1) * E],
                            start=(c == 0), stop=False)
                    nc.tensor.matmul(p[:], ones_sb[:], bcr_sb[:],
                                     start=False, stop=True)
                    nc.vector.tensor_copy(skip_sb[:, it * E:(it + 1) * E],
                                          p[:])

                # ---- overlap with AG: h_selfT = Wself^T @ H^T ----
                for c2 in range(2):
                    for i2 in range(2):
                        p = ps_mm.tile([128, 512], F32, tag="mm")
                        for c in range(2):
                            nc.tensor.matmul(
                                p[:],
                                wself_sb[:, c * E + c2 * 128:
                                         c * E + (c2 + 1) * 128],
                                ht_sb[:, c * NQ + i2 * 512:
                                      c * NQ + i2 * 512 + 512],
                                start=(c == 0), stop=(c == 1))
                        nc.vector.tensor_copy(
                            hselft_sb[:, c2 * NQ + i2 * 512:
                                      c2 * NQ + i2 * 512 + 512], p[:])

            # ======== phase 2: attention (mlp pool released) ========
            with tc.tile_pool(name="attn", bufs=1) as at_pool:
                kt_sb = at_pool.tile([128, 2 * N], ATT, tag="kt")
                vones_sb = at_pool.tile([128, 64 * E1], ATT, tag="vones")

                def load_gathered(h):
                    for r in range(NCORES):
                        for c in range(2):
                            nc.sync.dma_start(
                                kt_sb[:, c * N + r * NQ + h * 512:
                                      c * N + r * NQ + h * 512 + 512],
                                ag_out[h][r * 128:(r + 1) * 128,
                                          c * 512:(c + 1) * 512])
                    for r in range(NCORES):
                        nc.sync.dma_start(
                            vones_sb[:, (r * 8 + h * 4) * E1:
                                     (r * 8 + h * 4 + 4) * E1],
                            ag_out[h][r * 128:(r + 1) * 128, 1024:AGW])

                load_gathered(0)
                load_gathered(1)

                def st_quarter(b, ph, pt_sb):
                    # S^T+exp for 16 pairs (32 j-tiles) of phase ph
                    for pr in range(16):
                        p_s = ps_mm.tile([128, 512], F32, tag="mm")
                        for half in range(2):
                            j = JPERM[ph * 32 + 2 * pr + half]
                            for c in range(2):
                                nc.tensor.matmul(
                                    p_s[:, half * 256:half * 256 + 256],
                                    kt_sb[:, c * N + j * 128:
                                          c * N + (j + 1) * 128],
                                    qt_sb[:, c * NQ + b * 256:
                                          c * NQ + (b + 1) * 256],
                                    start=(c == 0), stop=(c == 1))
                        nc.scalar.activation(
                            pt_sb[:, pr * 512:(pr + 1) * 512],
                            p_s[:], AF.Exp, scale=SCALE)

                def av_quarter(b, ph, pt_sb):
                    # AV partial over this phase's 32 j-tiles
                    for ic in range(2):
                        it = b * 2 + ic
                        p_av = ps_av.tile([128, E1], F32, tag="av")
                        for q in range(32):
                            jg = JPERM[ph * 32 + q]
                            nc.tensor.matmul(
                                p_av[:],
                                pt_sb[:, q * E + ic * 128:
                                      q * E + ic * 128 + 128],
                                vones_sb[:, jg * E1:(jg + 1) * E1],
                                start=(q == 0), stop=(q == 31))
                        sl = attn_sb[:, it * E1:(it + 1) * E1]
                        if ph == 0:
                            nc.vector.tensor_copy(sl, p_av[:])
                        else:
                            nc.vector.tensor_add(sl, sl, p_av[:])

                hiddent_sb = at_pool.tile([128, 4 * NQ], F32R, tag="hiddent")

                def finalize(b, hf_pool):
                    for ic in range(2):
                        it = b * 2 + ic
                        nc.vector.reciprocal(
                            recip_sb[:, it:it + 1],
                            attn_sb[:, it * E1 + E:(it + 1) * E1])
                        hf_t = hf_pool.tile([128, E], F32R, tag="hf")
                        nc.vector.scalar_tensor_tensor(
                            hf_t[:], attn_sb[:, it * E1:it * E1 + E],
                            recip_sb[:, it:it + 1],
                            skip_sb[:, it * E:(it + 1) * E],
                            op0=mybir.AluOpType.mult,
                            op1=mybir.AluOpType.add)
                        for c in range(2):
                            p_tr = ps_tr.tile([128, 128], F32R, tag="tr")
                            nc.tensor.transpose(
                                p_tr[:], hf_t[:, c * 128:(c + 1) * 128],
                                id_sb[:])
                            nc.vector.tensor_copy(
                                hfuset_sb[:, c * NQ + it * 128:
                                          c * NQ + (it + 1) * 128],
                                p_tr[:])

                def head_half(i2):
                    # h_fuseT = Wctx^T @ H_fuse^T for this query half
                    for c2 in range(2):
                        p = ps_mm.tile([128, 512], F32, tag="mm")
                        for c in range(2):
                            nc.tensor.matmul(
                                p[:],
                                wctx_sb[:, c * E + c2 * 128:
                                        c * E + (c2 + 1) * 128],
                                hfuset_sb[:, c * NQ + i2 * 512:
                                          c * NQ + i2 * 512 + 512],
                                start=(c == 0), stop=(c == 1))
                        nc.vector.tensor_copy(
                            hctxt_sb[:, c2 * NQ + i2 * 512:
                                     c2 * NQ + i2 * 512 + 512], p[:])
                    for f in range(4):
                        p = ps_mm.tile([128, 512], F32, tag="mm")
                        for kc in range(4):
                            rhs_sb = hselft_sb if kc < 2 else hctxt_sb
                            nc.tensor.matmul(
                                p[:],
                                wf1_sb[:, kc * HID + f * 128:
                                       kc * HID + (f + 1) * 128],
                                rhs_sb[:, (kc % 2) * NQ + i2 * 512:
                                       (kc % 2) * NQ + i2 * 512 + 512],
                                start=(kc == 0), stop=(kc == 3))
                        nc.scalar.activation(
                            hiddent_sb[:, f * NQ + i2 * 512:
                                       f * NQ + i2 * 512 + 512],
                            p[:], AF.Relu, bias=bf1_sb[:, f:f + 1])
                    p = ps_mm.tile([128, 512], F32, tag="mm")
                    for kc in range(4):
                        nc.tensor.matmul(
                            p[0:LAT2, :],
                            wf2_sb[:, kc * LAT2:(kc + 1) * LAT2],
                            hiddent_sb[:, kc * NQ + i2 * 512:
                                       kc * NQ + i2 * 512 + 512],
                            start=(kc == 0), stop=(kc == 3))
                    nc.scalar.activation(
                        outt_sb[0:LAT2, i2 * 512:i2 * 512 + 512],
                        p[0:LAT2, :], AF.Identity, bias=bf2_sb[0:LAT2, :])
                    nc.sync.dma_start(outT[:, i2 * 512:i2 * 512 + 512],
                                      outt_sb[0:LAT2, i2 * 512:i2 * 512 + 512])

                with tc.tile_pool(name="pt", bufs=2) as pt_pool, \
                     tc.tile_pool(name="hf", bufs=3) as hf_pool:
                    # phase A: all blocks on AG0's j-tiles (hides AG1)
                    for b in range(4):
                        pt_sb = pt_pool.tile([128, 32 * E], ATT, tag="pt")
                        st_quarter(b, 0, pt_sb)
                        av_quarter(b, 0, pt_sb)
                    # phase B: AG1's j-tiles + finalize; head overlapped
                    # per query-half as its H_fuse^T tiles complete
                    for b in range(4):
                        pt_sb = pt_pool.tile([128, 32 * E], ATT, tag="pt")
                        st_quarter(b, 1, pt_sb)
                        av_quarter(b, 1, pt_sb)
                        finalize(b, hf_pool)
                        if b == 1:
                            head_half(0)
                    head_half(1)

    nc.compile()
    return nc


def _get_nc():
    if "nc" not in _cache:
        _cache["nc"] = _build()
    return _cache["nc"]


def kernel(x, W1, b1, W2, b2, W3, b3, Wq, bq, Wk, bk, Wv, bv,
           Wskip, bskip, Wself, Wctx, Wf1, bf1, Wf2, bf2,
           _trace=False, _tmpdir=None):
    nc = _get_nc()
    f32 = np.float32
    x = np.asarray(x, f32)
    xT_full = np.ascontiguousarray(x.T)                      # [1024, 8192]
    col = lambda v: np.ascontiguousarray(np.asarray(v, f32).reshape(-1, 1))
    bcomb = (np.asarray(bskip, f32) + np.asarray(bv, f32)).reshape(1, -1)
    shared = {
        "W1": np.ascontiguousarray(W1, f32),
        "W2": np.ascontiguousarray(W2, f32),
        "W3": np.ascontiguousarray(W3, f32),
        "Wq": np.ascontiguousarray(Wq, f32),
        "Wk": np.ascontiguousarray(Wk, f32),
        "Wv": np.ascontiguousarray(Wv, f32),
        "Wskip": np.ascontiguousarray(Wskip, f32),
        "Wself": np.ascontiguousarray(Wself, f32),
        "Wctx": np.ascontiguousarray(Wctx, f32),
        "Wf1": np.ascontiguousarray(Wf1, f32),
        "Wf2": np.ascontiguousarray(Wf2, f32),
        "b1": col(b1), "b2": col(b2), "b3": col(b3),
        "bq": col(bq), "bk": col(bk),
        "bf1": col(bf1), "bf2": col(bf2),
        "bcomb_row": np.ascontiguousarray(bcomb),
        "ones_row": np.ones((1, 128), f32),
        "ident": np.eye(128, dtype=f32),
    }
    in_maps = []
    for r in range(NCORES):
        m = dict(shared)
        m["xT"] = np.ascontiguousarray(xT_full[:, r * NQ:(r + 1) * NQ])
        in_maps.append(m)

    kwargs = {}
    if _trace:
        kwargs = dict(trace=True, tmpdir=_tmpdir)
    res = run_bass_kernel_spmd(nc, in_maps, core_ids=list(range(NCORES)),
                               **kwargs)
    out = np.empty((N, LAT2), f32)
    for r in range(NCORES):
        out[r * NQ:(r + 1) * NQ, :] = res.results[r]["outT"].T
    mu = np.ascontiguousarray(out[:, :LAT2 // 2])
    logvar = np.ascontiguousarray(out[:, LAT2 // 2:])
    if _trace:
        return (mu, logvar), res
    return mu, logvar


# revision 22
# speedup vs baseline: 1.1229x; 1.1007x over previous
"""Trainium2 Bass kernel for nn_EncoderTransformerConv (8-core SPMD).

MLP encoder + dense single-head TransformerConv attention + VAE head.
N=8192 nodes sharded 8 ways (1024 query rows/core); K/V all-gathered
in two pipelined halves; attention runs in two j-phases so the second
gather hides under the first phase's compute.

Layouts: activations feature-on-partition ("transposed") so every matmul
contracts over the partition dim with no on-chip input transposes.
float32r (full-rate fp32) for MLP/head; fp8e4m3 for the O(N^2) attention
operands (error contribution ~1e-3 total, gate is 2e-2).
"""
import sys

if '/opt/trn_rl_repo' not in sys.path:
    sys.path.insert(0, '/opt/trn_rl_repo')

import numpy as np
import ml_dtypes
_bf16np = ml_dtypes.bfloat16
import concourse.bass as bass
import concourse.mybir as mybir
import concourse.tile as tile
from concourse import bacc
from concourse.bass_utils import run_bass_kernel_spmd

dt = mybir.dt
F32 = dt.float32
F32R = dt.float32r
ATT = dt.float8e4          # attention operand dtype (e4m3)
BF16 = dt.bfloat16
AF = mybir.ActivationFunctionType

NCORES = 8
N = 8192
NQ = N // NCORES            # 1024 query rows per core
IN_DIM = 1024
HID = 512
E = 256                      # embed dim == head dim
E1 = E + 1                   # V columns + ones column
LAT2 = 32                    # 2 * latent
SCALE = 1.0 / 16.0           # 1/sqrt(E)

# j-tile order: all ranks' first-half tiles (covered by AG part 0), then
# all ranks' second-half tiles (AG part 1).
JPERM = ([r * 8 + jt for r in range(NCORES) for jt in range(4)] +
         [r * 8 + jt for r in range(NCORES) for jt in range(4, 8)])

_cache = {}


def _build():
    nc = bacc.Bacc("TRN2", target_bir_lowering=False, debug=False,
                   num_devices=NCORES)

    # ---- DRAM I/O ----
    xT = nc.dram_tensor("xT", [IN_DIM, NQ], F32R, kind="ExternalInput")
    W1 = nc.dram_tensor("W1", [IN_DIM, HID], F32R, kind="ExternalInput")
    W2 = nc.dram_tensor("W2", [HID, HID], F32R, kind="ExternalInput")
    W3 = nc.dram_tensor("W3", [HID, E], F32R, kind="ExternalInput")
    Wq = nc.dram_tensor("Wq", [E, E], F32R, kind="ExternalInput")
    Wk = nc.dram_tensor("Wk", [E, E], F32R, kind="ExternalInput")
    Wv = nc.dram_tensor("Wv", [E, E], F32R, kind="ExternalInput")
    Wskip = nc.dram_tensor("Wskip", [E, E], F32R, kind="ExternalInput")
    Wself = nc.dram_tensor("Wself", [E, E], F32R, kind="ExternalInput")
    Wctx = nc.dram_tensor("Wctx", [E, E], F32R, kind="ExternalInput")
    Wf1 = nc.dram_tensor("Wf1", [2 * E, HID], F32R, kind="ExternalInput")
    Wf2 = nc.dram_tensor("Wf2", [HID, LAT2], F32R, kind="ExternalInput")
    b1 = nc.dram_tensor("b1", [HID, 1], F32, kind="ExternalInput")
    b2 = nc.dram_tensor("b2", [HID, 1], F32, kind="ExternalInput")
    b3 = nc.dram_tensor("b3", [E, 1], F32, kind="ExternalInput")
    bq = nc.dram_tensor("bq", [E, 1], F32, kind="ExternalInput")
    bk = nc.dram_tensor("bk", [E, 1], F32, kind="ExternalInput")
    bf1 = nc.dram_tensor("bf1", [HID, 1], F32, kind="ExternalInput")
    bf2 = nc.dram_tensor("bf2", [LAT2, 1], F32, kind="ExternalInput")
    bcomb_row = nc.dram_tensor("bcomb_row", [1, E], F32R, kind="ExternalInput")
    ones_row = nc.dram_tensor("ones_row", [1, 128], F32R, kind="ExternalInput")
    ident = nc.dram_tensor("ident", [128, 128], F32R, kind="ExternalInput")
    outT = nc.dram_tensor("outT", [LAT2, NQ], F32, kind="ExternalOutput")

    def chunked_load(sb_t, dram_t, cols):
        nc.sync.dma_start(sb_t[:].rearrange("p (k f) -> p k f", f=cols),
                          dram_t[:].rearrange("(k p) f -> p k f", p=128))

    def bias_load(sb_t, dram_t):
        nc.sync.dma_start(sb_t[:].rearrange("p (k o) -> p k o", o=1),
                          dram_t[:].rearrange("(k p) o -> p k o", p=128))

    with tile.TileContext(nc) as tc:
        with tc.tile_pool(name="persist", bufs=1) as pe_pool, \
             tc.tile_pool(name="ps_mm", bufs=4, space="PSUM") as ps_mm, \
             tc.tile_pool(name="ps_av", bufs=2, space="PSUM") as ps_av, \
             tc.tile_pool(name="ps_tr", bufs=2, space="PSUM") as ps_tr, \
             tc.tile_pool(name="dram", bufs=1, space="DRAM") as dram:

            # ---- persistent SBUF ----
            wq_sb = pe_pool.tile([128, 2 * E], F32R, tag="wq")
            wk_sb = pe_pool.tile([128, 2 * E], F32R, tag="wk")
            wv_sb = pe_pool.tile([128, 2 * E], F32R, tag="wv")
            wskip_sb = pe_pool.tile([128, 2 * E], F32R, tag="wskip")
            wself_sb = pe_pool.tile([128, 2 * E], F32R, tag="wself")
            wctx_sb = pe_pool.tile([128, 2 * E], F32R, tag="wctx")
            wf1_sb = pe_pool.tile([128, 4 * HID], F32R, tag="wf1")
            wf2_sb = pe_pool.tile([128, 4 * LAT2], F32R, tag="wf2")
            b1_sb = pe_pool.tile([128, 4], F32, tag="b1")
            b2_sb = pe_pool.tile([128, 4], F32, tag="b2")
            b3_sb = pe_pool.tile([128, 2], F32, tag="b3")
            bq_sb = pe_pool.tile([128, 2], F32, tag="bq")
            bk_sb = pe_pool.tile([128, 2], F32, tag="bk")
            bf1_sb = pe_pool.tile([128, 4], F32, tag="bf1")
            bf2_sb = pe_pool.tile([128, 1], F32, tag="bf2")
            bcr_sb = pe_pool.tile([1, E], F32R, tag="bcr")
            ones_sb = pe_pool.tile([1, 128], F32R, tag="ones")
            id_sb = pe_pool.tile([128, 128], F32R, tag="ident")
            ht_sb = pe_pool.tile([128, 2 * NQ], F32R, tag="ht")
            qt_sb = pe_pool.tile([128, 2 * NQ], ATT, tag="qt")
            skip_sb = pe_pool.tile([128, 8 * E], F32, tag="skip")
            hselft_sb = pe_pool.tile([128, 2 * NQ], F32R, tag="hselft")
            hfuset_sb = pe_pool.tile([128, 2 * NQ], F32R, tag="hfuset")
            hctxt_sb = pe_pool.tile([128, 2 * NQ], F32R, tag="hctxt")
            recip_sb = pe_pool.tile([128, 8], F32, tag="recip")
            attn_sb = pe_pool.tile([128, 8 * E1], F32, tag="attn")
            outt_sb = pe_pool.tile([128, NQ], F32, tag="outt")

            # AG bounce buffers: per half, [c0 512 | c1 512 | V(4x257)] fp8
            AGW = 2 * 512 + 4 * E1        # 2052 fp8 columns
            ag_in0 = dram.tile([128, AGW], ATT, tag="agi0")
            ag_in1 = dram.tile([128, AGW], ATT, tag="agi1")
            ag_in = [ag_in0, ag_in1]
            ag_out = [nc.dram_tensor(f"ag_out{h}", [NCORES * 128, AGW],
                                     ATT, kind="Internal",
                                     addr_space="Shared")
                      for h in range(2)]

            # ======== phase 1: MLP + projections + split AllGather ========
            with tc.tile_pool(name="mlp", bufs=1) as mlp_pool, \
                 tc.tile_pool(name="xt", bufs=16) as xt_pool:
                w1_sb = mlp_pool.tile([128, 8 * HID], F32R, tag="w1")
                w2_sb = mlp_pool.tile([128, 4 * HID], F32R, tag="w2")
                w3_sb = mlp_pool.tile([128, 4 * E], F32R, tag="w3")
                h1t_sb = mlp_pool.tile([128, 4 * NQ], F32R, tag="h1t")
                h2t_sb = mlp_pool.tile([128, 4 * NQ], F32R, tag="h2t")
                kts_sb = mlp_pool.tile([128, 2 * NQ], ATT, tag="kts")
                vs_sb = mlp_pool.tile([128, 8 * E1], ATT, tag="vs")

                def load_x(i2, interleave_w1):
                    # per-chunk loads; optionally interleave W1 chunk loads
                    # so the k-outer h1 matmuls can start after chunk 0
                    xts = []
                    for k in range(8):
                        if interleave_w1:
                            nc.sync.dma_start(
                                w1_sb[:, k * HID:(k + 1) * HID],
                                W1[k * 128:(k + 1) * 128, :])
                        xt_t = xt_pool.tile([128, 512], F32R, tag="xt")
                        nc.sync.dma_start(
                            xt_t[:],
                            xT[k * 128:(k + 1) * 128,
                               i2 * 512:(i2 + 1) * 512])
                        xts.append(xt_t)
                    return xts

                def h1_half(i2, xts):
                    # k-outer: 4 PSUM groups held; streams x chunk-by-chunk
                    ps_h1 = []
                    for _f in range(4):
                        ph1_t = ps_mm.tile([128, 512], F32, tag="mm")
                        ps_h1.append(ph1_t)
                    for k in range(8):
                        for f in range(4):
                            nc.tensor.matmul(
                                ps_h1[f][:],
                                w1_sb[:, k * HID + f * 128:
                                      k * HID + (f + 1) * 128],
                                xts[k][:], start=(k == 0), stop=(k == 7))
                    for f in range(4):
                        nc.scalar.activation(
                            h1t_sb[:, f * NQ + i2 * 512:
                                   f * NQ + i2 * 512 + 512],
                            ps_h1[f][:], AF.Relu, bias=b1_sb[:, f:f + 1])

                def mlp_half(i2):
                    for f in range(4):
                        p = ps_mm.tile([128, 512], F32, tag="mm")
                        for k in range(4):
                            nc.tensor.matmul(
                                p[:],
                                w2_sb[:, k * HID + f * 128:
                                      k * HID + (f + 1) * 128],
                                h1t_sb[:, k * NQ + i2 * 512:
                                       k * NQ + i2 * 512 + 512],
                                start=(k == 0), stop=(k == 3))
                        nc.scalar.activation(
                            h2t_sb[:, f * NQ + i2 * 512:
                                   f * NQ + i2 * 512 + 512],
                            p[:], AF.Relu, bias=b2_sb[:, f:f + 1])
                    for c in range(2):
                        p = ps_mm.tile([128, 512], F32, tag="mm")
                        for k in range(4):
                            nc.tensor.matmul(
                                p[:],
                                w3_sb[:, k * E + c * 128:
                                      k * E + (c + 1) * 128],
                                h2t_sb[:, k * NQ + i2 * 512:
                                       k * NQ + i2 * 512 + 512],
                                start=(k == 0), stop=(k == 3))
                        nc.scalar.activation(
                            ht_sb[:, c * NQ + i2 * 512:
                                  c * NQ + i2 * 512 + 512],
                            p[:], AF.Relu, bias=b3_sb[:, c:c + 1])

                def kv_half(i2):
                    # K^T columns for this half (+bk), fp8
                    for c2 in range(2):
                        p = ps_mm.tile([128, 512], F32, tag="mm")
                        for c in range(2):
                            nc.tensor.matmul(
                                p[:],
                                wk_sb[:, c * E + c2 * 128:
                                      c * E + (c2 + 1) * 128],
                                ht_sb[:, c * NQ + i2 * 512:
                                      c * NQ + i2 * 512 + 512],
                                start=(c == 0), stop=(c == 1))
                        nc.vector.tensor_scalar_add(
                            kts_sb[:, c2 * NQ + i2 * 512:
                                   c2 * NQ + i2 * 512 + 512],
                            p[:], bk_sb[:, c2:c2 + 1])
                    # V rows for this half (bias bv folded into bcomb)
                    for jt in range(i2 * 4, i2 * 4 + 4):
                        p = ps_mm.tile([128, 256], F32, tag="mm")
                        for c in range(2):
                            nc.tensor.matmul(
                                p[:],
                                ht_sb[:, c * NQ + jt * 128:
                                      c * NQ + (jt + 1) * 128],
                                wv_sb[:, c * E:(c + 1) * E],
                                start=(c == 0), stop=(c == 1))
                        nc.vector.tensor_copy(
                            vs_sb[:, jt * E1:jt * E1 + E], p[:])
                    nc.vector.memset(
                        vs_sb[:, i2 * 4 * E1:(i2 * 4 + 4) * E1].rearrange(
                            "p (j c) -> p j c", c=E1)[:, :, E:E1], 1.0)
                    # bounce + AllGather this half
                    nc.scalar.dma_start(ag_in[i2][:, 0:512],
                                        kts_sb[:, i2 * 512:i2 * 512 + 512])
                    nc.scalar.dma_start(ag_in[i2][:, 512:1024],
                                        kts_sb[:, NQ + i2 * 512:
                                               NQ + i2 * 512 + 512])
                    nc.scalar.dma_start(ag_in[i2][:, 1024:AGW],
                                        vs_sb[:, i2 * 4 * E1:(i2 * 4 + 4) * E1])
                    nc.gpsimd.collective_compute(
                        "AllGather", mybir.AluOpType.bypass,
                        replica_groups=[list(range(NCORES))],
                        ins=[ag_in[i2][:].opt()],
                        outs=[ag_out[i2][:].opt()],
                    )

                # identity first (tiny), then warmup matmuls keep the
                # PE HAM clock-gate busy (-> 2.4 GHz) while W1/x stream in
                nc.sync.dma_start(id_sb[:], ident[:])
                warm_ps = ps_av.tile([128, E1], F32, tag="av")
                for _w in range(20):
                    nc.tensor.matmul(warm_ps[:, 0:128], id_sb[:], id_sb[:],
                                     start=True, stop=True)
                bias_load(b1_sb, b1)
                xts0 = load_x(0, interleave_w1=True)
                # remaining MLP weights behind W1/x0 in queue order
                chunked_load(w2_sb, W2, HID)
                chunked_load(w3_sb, W3, E)
                chunked_load(wk_sb, Wk, E)
                chunked_load(wv_sb, Wv, E)
                bias_load(b2_sb, b2)
                bias_load(b3_sb, b3)
                bias_load(bk_sb, bk)
                h1_half(0, xts0)
                mlp_half(0)
                kv_half(0)
                xts1 = load_x(1, interleave_w1=False)
                # attention/head weights prefetched during half-0 compute
                chunked_load(wq_sb, Wq, E)
                chunked_load(wskip_sb, Wskip, E)
                chunked_load(wself_sb, Wself, E)
                chunked_load(wctx_sb, Wctx, E)
                chunked_load(wf1_sb, Wf1, HID)
                chunked_load(wf2_sb, Wf2, LAT2)
                bias_load(bq_sb, bq)
                bias_load(bf1_sb, bf1)
                nc.sync.dma_start(bf2_sb[0:LAT2, :], bf2[:])
                nc.sync.dma_start(bcr_sb[:], bcomb_row[:])
                nc.sync.dma_start(ones_sb[:], ones_row[:])
                h1_half(1, xts1)
                mlp_half(1)
                kv_half(1)

                # ---- overlap with AG: Q^T (+bq, fp8) ----
                for c2 in range(2):
                    for i2 in range(2):
                        p = ps_mm.tile([128, 512], F32, tag="mm")
                        for c in range(2):
                            nc.tensor.matmul(
                                p[:],
                                wq_sb[:, c * E + c2 * 128:
                                      c * E + (c2 + 1) * 128],
                                ht_sb[:, c * NQ + i2 * 512:
                                      c * NQ + i2 * 512 + 512],
                                start=(c == 0), stop=(c == 1))
                        nc.vector.tensor_scalar_add(
                            qt_sb[:, c2 * NQ + i2 * 512:
                                  c2 * NQ + i2 * 512 + 512],
                            p[:], bq_sb[:, c2:c2 + 1])

                # ---- overlap with AG: skip = H@Wskip + (bskip+bv) ----
                for it in range(8):
                    p = ps_mm.tile([128, 256], F32, tag="mm")
                    for c in range(2):
                        nc.tensor.matmul(
                            p[:],
                            ht_sb[:, c * NQ + it * 128:
                                  c * NQ + (it + 1) * 128],
                            wskip_sb[:, c * E:(c + 1) * E],
                            start=(c == 0), stop=False)
                    nc.tensor.matmul(p[:], ones_sb[:], bcr_sb[:],
                                     start=False, stop=True)
                    nc.vector.tensor_copy(skip_sb[:, it * E:(it + 1) * E],
                                          p[:])

                # ---- overlap with AG: h_selfT = Wself^T @ H^T ----
                for c2 in range(2):
                    for i2 in range(2):
                        p = ps_mm.tile([128, 512], F32, tag="mm")
                        for c in range(2):
                            nc.tensor.matmul(
                                p[:],
                                wself_sb[:, c * E + c2 * 128:
                                         c * E + (c2 + 1) * 128],
                                ht_sb[:, c * NQ + i2 * 512:
                                      c * NQ + i2 * 512 + 512],
                                start=(c == 0), stop=(c == 1))
                        nc.vector.tensor_copy(
                            hselft_sb[:, c2 * NQ + i2 * 512:
                                      c2 * NQ + i2 * 512 + 512], p[:])

            # ======== phase 2: attention (mlp pool released) ========
            with tc.tile_pool(name="attn", bufs=1) as at_pool:
                kt_sb = at_pool.tile([128, 2 * N], ATT, tag="kt")
                vones_sb = at_pool.tile([128, 64 * E1], ATT, tag="vones")

                def load_gathered(h):
                    for r in range(NCORES):
                        for c in range(2):
                            nc.sync.dma_start(
                                kt_sb[:, c * N + r * NQ + h * 512:
                                      c * N + r * NQ + h * 512 + 512],
                                ag_out[h][r * 128:(r + 1) * 128,
                                          c * 512:(c + 1) * 512])
                    for r in range(NCORES):
                        nc.sync.dma_start(
                            vones_sb[:, (r * 8 + h * 4) * E1:
                                     (r * 8 + h * 4 + 4) * E1],
                            ag_out[h][r * 128:(r + 1) * 128, 1024:AGW])

                load_gathered(0)
                load_gathered(1)

                kt_v = kt_sb[:].rearrange("p (two n) -> p two n", two=2)
                qt_v = qt_sb[:].rearrange("p (two n) -> p two n", two=2)

                def st_quarter(b, ph, pt_sb):
                    # S^T+exp for 16 pairs (32 j-tiles) of phase ph.
                    # DoubleRow fp8: both 128-deep c-chunks contracted in
                    # one matmul (2 weights/cell).
                    for pr in range(16):
                        p_s = ps_mm.tile([128, 512], F32, tag="mm")
                        for half in range(2):
                            j = JPERM[ph * 32 + 2 * pr + half]
                            nc.tensor.matmul(
                                p_s[:, half * 256:half * 256 + 256],
                                kt_v[:, :, j * 128:(j + 1) * 128],
                                qt_v[:, :, b * 256:(b + 1) * 256],
                                start=True, stop=True,
                                perf_mode=mybir.MatmulPerfMode.DoubleRow)
                        nc.scalar.activation(
                            pt_sb[:, pr * 512:(pr + 1) * 512],
                            p_s[:], AF.Exp, scale=SCALE)

                def av_quarter(b, ph, pt_sb):
                    # AV partial over this phase's 32 j-tiles
                    for ic in range(2):
                        it = b * 2 + ic
                        p_av = ps_av.tile([128, E1], F32, tag="av")
                        for q in range(32):
                            jg = JPERM[ph * 32 + q]
                            nc.tensor.matmul(
                                p_av[:],
                                pt_sb[:, q * E + ic * 128:
                                      q * E + ic * 128 + 128],
                                vones_sb[:, jg * E1:(jg + 1) * E1],
                                start=(q == 0), stop=(q == 31))
                        sl = attn_sb[:, it * E1:(it + 1) * E1]
                        if ph == 0:
                            nc.vector.tensor_copy(sl, p_av[:])
                        else:
                            nc.vector.tensor_add(sl, sl, p_av[:])

                hiddent_sb = at_pool.tile([128, 4 * NQ], F32R, tag="hiddent")

                def finalize(b, hf_pool):
                    for ic in range(2):
                        it = b * 2 + ic
                        nc.vector.reciprocal(
                            recip_sb[:, it:it + 1],
                            attn_sb[:, it * E1 + E:(it + 1) * E1])
                        hf_t = hf_pool.tile([128, E], F32R, tag="hf")
                        nc.vector.scalar_tensor_tensor(
                            hf_t[:], attn_sb[:, it * E1:it * E1 + E],
                            recip_sb[:, it:it + 1],
                            skip_sb[:, it * E:(it + 1) * E],
                            op0=mybir.AluOpType.mult,
                            op1=mybir.AluOpType.add)
                        for c in range(2):
                            p_tr = ps_tr.tile([128, 128], F32R, tag="tr")
                            nc.tensor.transpose(
                                p_tr[:], hf_t[:, c * 128:(c + 1) * 128],
                                id_sb[:])
                            nc.vector.tensor_copy(
                                hfuset_sb[:, c * NQ + it * 128:
                                          c * NQ + (it + 1) * 128],
                                p_tr[:])

                def head_half(i2):
                    # h_fuseT = Wctx^T @ H_fuse^T for this query half
                    for c2 in range(2):
                        p = ps_mm.tile([128, 512], F32, tag="mm")
                        for c in range(2):
                            nc.tensor.matmul(
                                p[:],
                                wctx_sb[:, c * E + c2 * 128:
                                        c * E + (c2 + 1) * 128],
                                hfuset_sb[:, c * NQ + i2 * 512:
                                          c * NQ + i2 * 512 + 512],
                                start=(c == 0), stop=(c == 1))
                        nc.vector.tensor_copy(
                            hctxt_sb[:, c2 * NQ + i2 * 512:
                                     c2 * NQ + i2 * 512 + 512], p[:])
                    for f in range(4):
                        p = ps_mm.tile([128, 512], F32, tag="mm")
                        for kc in range(4):
                            rhs_sb = hselft_sb if kc < 2 else hctxt_sb
                            nc.tensor.matmul(
                                p[:],
                                wf1_sb[:, kc * HID + f * 128:
                                       kc * HID + (f + 1) * 128],
                                rhs_sb[:, (kc % 2) * NQ + i2 * 512:
                                       (kc % 2) * NQ + i2 * 512 + 512],
                                start=(kc == 0), stop=(kc == 3))
                        nc.scalar.activation(
                            hiddent_sb[:, f * NQ + i2 * 512:
                                       f * NQ + i2 * 512 + 512],
                            p[:], AF.Relu, bias=bf1_sb[:, f:f + 1])
                    p = ps_mm.tile([128, 512], F32, tag="mm")
                    for kc in range(4):
                        nc.tensor.matmul(
                            p[0:LAT2, :],
                            wf2_sb[:, kc * LAT2:(kc + 1) * LAT2],
                            hiddent_sb[:, kc * NQ + i2 * 512:
                                       kc * NQ + i2 * 512 + 512],
                            start=(kc == 0), stop=(kc == 3))
                    nc.scalar.activation(
                        outt_sb[0:LAT2, i2 * 512:i2 * 512 + 512],
                        p[0:LAT2, :], AF.Identity, bias=bf2_sb[0:LAT2, :])
                    nc.sync.dma_start(outT[:, i2 * 512:i2 * 512 + 512],
                                      outt_sb[0:LAT2, i2 * 512:i2 * 512 + 512])

                with tc.tile_pool(name="pt", bufs=2) as pt_pool, \
                     tc.tile_pool(name="hf", bufs=3) as hf_pool:
                    # phase A: all blocks on AG0's j-tiles (hides AG1)
                    for b in range(4):
                        pt_sb = pt_pool.tile([128, 32 * E], ATT, tag="pt")
                        st_quarter(b, 0, pt_sb)
                        av_quarter(b, 0, pt_sb)
                    # phase B: AG1's j-tiles + finalize; head overlapped
                    # per query-half as its H_fuse^T tiles complete
                    for b in range(4):
                        pt_sb = pt_pool.tile([128, 32 * E], ATT, tag="pt")
                        st_quarter(b, 1, pt_sb)
                        av_quarter(b, 1, pt_sb)
                        finalize(b, hf_pool)
                        if b == 1:
                            head_half(0)
                    head_half(1)

    nc.compile()
    return nc


def _get_nc():
    if "nc" not in _cache:
        _cache["nc"] = _build()
    return _cache["nc"]


def kernel(x, W1, b1, W2, b2, W3, b3, Wq, bq, Wk, bk, Wv, bv,
           Wskip, bskip, Wself, Wctx, Wf1, bf1, Wf2, bf2,
           _trace=False, _tmpdir=None):
    nc = _get_nc()
    f32 = np.float32
    x = np.asarray(x, f32)
    xT_full = np.ascontiguousarray(x.T)                      # [1024, 8192]
    col = lambda v: np.ascontiguousarray(np.asarray(v, f32).reshape(-1, 1))
    bcomb = (np.asarray(bskip, f32) + np.asarray(bv, f32)).reshape(1, -1)
    shared = {
        "W1": np.ascontiguousarray(W1, f32),
        "W2": np.ascontiguousarray(W2, f32),
        "W3": np.ascontiguousarray(W3, f32),
        "Wq": np.ascontiguousarray(Wq, f32),
        "Wk": np.ascontiguousarray(Wk, f32),
        "Wv": np.ascontiguousarray(Wv, f32),
        "Wskip": np.ascontiguousarray(Wskip, f32),
        "Wself": np.ascontiguousarray(Wself, f32),
        "Wctx": np.ascontiguousarray(Wctx, f32),
        "Wf1": np.ascontiguousarray(Wf1, f32),
        "Wf2": np.ascontiguousarray(Wf2, f32),
        "b1": col(b1), "b2": col(b2), "b3": col(b3),
        "bq": col(bq), "bk": col(bk),
        "bf1": col(bf1), "bf2": col(bf2),
        "bcomb_row": np.ascontiguousarray(bcomb),
        "ones_row": np.ones((1, 128), f32),
        "ident": np.eye(128, dtype=f32),
    }
    in_maps = []
    for r in range(NCORES):
        m = dict(shared)
        m["xT"] = np.ascontiguousarray(xT_full[:, r * NQ:(r + 1) * NQ])
        in_maps.append(m)

    kwargs = {}
    if _trace:
        kwargs = dict(trace=True, tmpdir=_tmpdir)
    res = run_bass_kernel_spmd(nc, in_maps, core_ids=list(range(NCORES)),
                               **kwargs)
    out = np.empty((N, LAT2), f32)
    for r in range(NCORES):
        out[r * NQ:(r + 1) * NQ, :] = res.results[r]["outT"].T
    mu = np.ascontiguousarray(out[:, :LAT2 // 2])
    logvar = np.ascontiguousarray(out[:, LAT2 // 2:])
    if _trace:
        return (mu, logvar), res
    return mu, logvar
